# revision 1
# baseline (speedup 1.0000x reference)
"""GF(2) linear block encoder c = (b @ G) mod 2 on 8 TRN2 NeuronCores.

Strategy:
  - Data-parallel: shard b rows (32768 -> 8 x 4096), replicate G.
  - Bits {0,1} are exact in fp8-e4m3 and products accumulate exactly in
    fp32 PSUM (sums <= 1024 << 2^24), so the GF(2) matmul is computed as
    an fp8 DoubleRow matmul (K=256 per MM) at ~2x bf16 throughput.
  - mod 2 is a single DVE tensor_scalar(mod, 2.0) pass PSUM->SBUF uint8.
  - Host packs b into [128, 8, M] (k = s*128 + p) transposed layout and
    casts to fp8; output uint8 is cast back to int32 on host.
"""

import os
import sys

import numpy as np

if "/opt/trn_rl_repo" not in sys.path:
    sys.path.insert(0, "/opt/trn_rl_repo")

import ml_dtypes

B_ROWS = 32768
K_MSG = 1024
N_CODE = 2048
NCORES = 8
M = B_ROWS // NCORES  # 4096 rows per core
KS = K_MSG // 128     # 8 k-subtiles of 128
KP = KS // 2          # 4 DoubleRow k-pair steps (K=256 each)
MT = M // 128         # 32 m-tiles
NT = N_CODE // 512    # 4 n-tiles
MC = 8                # b DMA chunks along m
MCW = M // MC         # 512 m per chunk

F8 = ml_dtypes.float8_e4m3

_NC_CACHE = None


def _build_bass():
    import concourse.bacc as bacc
    import concourse.mybir as mybir
    from concourse import tile

    nc = bacc.Bacc("TRN2", target_bir_lowering=False, debug=False)

    bt = nc.dram_tensor("bt", [128, KS, M], mybir.dt.float8e4, kind="ExternalInput")
    g = nc.dram_tensor("g", [128, KS, N_CODE], mybir.dt.float8e4, kind="ExternalInput")
    c = nc.dram_tensor("c", [M, N_CODE], mybir.dt.int32, kind="ExternalOutput")

    dr = mybir.MatmulPerfMode.DoubleRow

    with tile.TileContext(nc) as tc:
        with (
            tc.tile_pool(name="persist", bufs=1) as persist,
            tc.tile_pool(name="psum", bufs=2, space="PSUM") as psum_pool,
            tc.tile_pool(name="couts", bufs=4) as couts,
        ):
            # alternate input DMAs between the two HWDGE queues (SP + ACT)
            # so input loading runs at 2x single-queue bandwidth
            in_engines = [nc.sync, nc.scalar]

            # G resident: 4 chunks of [128, 2, N] (k-pair each), contiguous DMA
            g_tiles = []
            for kp in range(KP):
                gt = persist.tile([128, 2, N_CODE], mybir.dt.float8e4, tag=f"g{kp}")
                in_engines[kp % 2].dma_start(out=gt, in_=g[:, 2 * kp : 2 * kp + 2, :])
                g_tiles.append(gt)

            # b resident: 8 chunks of [128, KS, 512] along m so compute can
            # start after the first chunk lands
            b_tiles = []
            for mc in range(MC):
                btile = persist.tile([128, KS, MCW], mybir.dt.float8e4, tag=f"b{mc}")
                in_engines[mc % 2].dma_start(
                    out=btile, in_=bt[:, :, mc * MCW : (mc + 1) * MCW]
                )
                b_tiles.append(btile)

            # output viewed as [MC groups, 128 p, 4 j, N]: row m = mc*512+j*128+p
            c_view = c.rearrange("(mc j p) n -> mc p j n", j=MT // MC, p=128)
            JT = MT // MC  # 4 m-tiles per output group

            for mt in range(MT):
                mc = mt // JT
                j = mt % JT
                m0 = j * 128
                if j == 0:
                    c_sb = couts.tile([128, JT, N_CODE], mybir.dt.int32)
                ps = psum_pool.tile([128, N_CODE], mybir.dt.float32)  # 4 banks
                for nt in range(NT):
                    for kp in range(KP):
                        nc.tensor.matmul(
                            ps[:, nt * 512 : (nt + 1) * 512],
                            b_tiles[mc][:, 2 * kp : 2 * kp + 2, m0 : m0 + 128],
                            g_tiles[kp][:, :, nt * 512 : (nt + 1) * 512],
                            start=(kp == 0),
                            stop=(kp == KP - 1),
                            perf_mode=dr,
                        )
                # mod 2 = LSB: one big ACT cast psum fp32 -> int32, then one
                # in-place DVE and-with-1 (big tiles amortize per-inst cost)
                nc.scalar.activation(
                    c_sb[:, j, :], ps, mybir.ActivationFunctionType.Copy
                )
                nc.vector.tensor_scalar(
                    out=c_sb[:, j, :],
                    in0=c_sb[:, j, :],
                    scalar1=1,
                    scalar2=None,
                    op0=mybir.AluOpType.bitwise_and,
                )
                if j == JT - 1:
                    # spread out-DMAs over three queues (SWDGE + both HWDGE)
                    # so the 32MB of output streams ~3x wider and the tail
                    # after the last compute shrinks accordingly
                    out_eng = (nc.gpsimd, nc.sync, nc.scalar)[mc % 3]
                    out_eng.dma_start(out=c_view[mc], in_=c_sb)

    nc.finalize()  # bacc: regalloc + event-semaphore legalization
    return nc


def _get_nc():
    global _NC_CACHE
    if _NC_CACHE is None:
        _NC_CACHE = _build_bass()
    return _NC_CACHE


def _pack_inputs(b, G):
    b8 = np.asarray(b).astype(np.uint8)
    G8 = np.asarray(G).astype(np.uint8)
    # [p, s, n] with k = s*128 + p
    g_f8 = G8.reshape(KS, 128, N_CODE).transpose(1, 0, 2).astype(F8, order="C")
    bts = []
    for core in range(NCORES):
        sh = b8[core * M : (core + 1) * M]          # [M, K]
        bt = sh.T.reshape(KS, 128, M).transpose(1, 0, 2)  # [p, s, m]
        bts.append(bt.astype(F8, order="C"))
    return bts, g_f8


def kernel(b, G, trace=False, **run_kwargs):
    from concourse.bass_utils import run_bass_kernel_spmd

    nc = _get_nc()
    bts, g_f8 = _pack_inputs(b, G)
    in_maps = [{"bt": bts[i], "g": g_f8} for i in range(NCORES)]
    res = run_bass_kernel_spmd(
        nc, in_maps, core_ids=list(range(NCORES)), trace=trace, **run_kwargs
    )
    out = np.concatenate([res.results[i]["c"] for i in range(NCORES)], axis=0)
    if out.dtype != np.int32:
        out = out.astype(np.int32)
    if trace:
        kernel.last_results = res
    return out


kernel.last_results = None



# revision 9
# speedup vs baseline: 1.1145x; 1.1145x over previous
"""GF(2) linear block encoder c = (b @ G) mod 2 on 8 TRN2 NeuronCores.

Strategy:
  - Data-parallel: shard b rows (32768 -> 8 x 4096), replicate G.
  - Bits {0,1} are exact in fp8-e4m3 and products accumulate exactly in
    fp32 PSUM, so the GF(2) matmul is an fp8 DoubleRow matmul (K=256 per
    MM) at 2x bf16 throughput -- the PE floor for this shape (~110us).
  - Output is written as uint8 bits (mod-2 extracted from PSUM by the
    DVE/Pool engines) and upcast to int32 on the host: 4x less output
    HBM traffic than int32, which removes the output-DMA tail.
  - Input DMAs are ordered so the first matmul's operands (G k-pair 0,
    b chunk 0) land first; the matmul loop is k-outer per m-tile so PE
    starts as soon as those arrive instead of after all 6 MiB of input.
"""

import sys

import numpy as np

if "/opt/trn_rl_repo" not in sys.path:
    sys.path.insert(0, "/opt/trn_rl_repo")

import ml_dtypes

B_ROWS = 32768
K_MSG = 1024
N_CODE = 2048
NCORES = 8
M = B_ROWS // NCORES  # 4096 rows per core
KS = K_MSG // 128     # 8 k-subtiles of 128
KP = KS // 2          # 4 DoubleRow k-pair steps (K=256 each)
MT = M // 128         # 32 m-tiles
NT = N_CODE // 512    # 4 n-chunks (one PSUM bank each)
MC = 16               # b DMA chunks along m (2 m-tiles each)
MCW = M // MC         # 256 rows per chunk

F8 = ml_dtypes.float8_e4m3

_NC_CACHE = None


def _build_bass():
    import concourse.bacc as bacc
    import concourse.mybir as mybir
    from concourse import tile

    nc = bacc.Bacc("TRN2", target_bir_lowering=False, debug=False)

    # bt[p, c, s, j] = b bit for row m = c*MCW + j, k = s*128 + p
    bt = nc.dram_tensor("bt", [128, MC, KS, MCW], mybir.dt.float8e4, kind="ExternalInput")
    g = nc.dram_tensor("g", [128, KS, N_CODE], mybir.dt.float8e4, kind="ExternalInput")
    c = nc.dram_tensor("c", [M, N_CODE], mybir.dt.uint16, kind="ExternalOutput")

    dr = mybir.MatmulPerfMode.DoubleRow

    with tile.TileContext(nc) as tc:
        with (
            tc.tile_pool(name="persist", bufs=1) as persist,
            tc.tile_pool(name="psum", bufs=2, space="PSUM") as psum_pool,
            tc.tile_pool(name="mids", bufs=4) as mids,
        ):
            # --- input DMAs, ordered for earliest PE start ---
            # sync:   g0, b0, g2, then odd b chunks
            # scalar: g1, g3, then even b chunks
            g_tiles = [
                persist.tile([128, 2, N_CODE], mybir.dt.float8e4, name=f"gt{kp}", tag=f"g{kp}")
                for kp in range(KP)
            ]
            b_tiles = [
                persist.tile([128, KS, MCW], mybir.dt.float8e4, name=f"btile{mc}", tag=f"b{mc}")
                for mc in range(MC)
            ]

            def load_g(kp, eng):
                eng.dma_start(out=g_tiles[kp], in_=g[:, 2 * kp : 2 * kp + 2, :])

            def load_b(mc, eng):
                eng.dma_start(out=b_tiles[mc], in_=bt[:, mc, :, :])

            load_g(0, nc.sync)
            load_g(1, nc.scalar)
            load_b(0, nc.sync)
            load_g(3, nc.scalar)
            load_g(2, nc.sync)
            load_b(1, nc.scalar)
            rr = [nc.sync, nc.scalar]
            for mc in range(2, MC):
                load_b(mc, rr[mc % 2])

            # output viewed per m-tile: m = mt*128 + p
            c_view = c.rearrange("(mt p) n -> mt p n", p=128)

            # mod-2 = LSB: ACT casts PSUM fp32 -> uint16 SBUF (exact, sums
            # <= 1024), then DVE does an in-place and-with-1 (Pool lacks
            # tensor_scalar, and only ACT/DVE can read PSUM)
            ext_engines = [nc.vector, nc.vector]
            # out-DMA queues: first tiles go to SWDGE (HWDGE queues still
            # carry input b chunks then), later tiles alternate sync/scalar
            out_eng = [nc.gpsimd] * 4 + [
                (nc.sync, nc.scalar)[i % 2] for i in range(MT - 4)
            ]

            for mt in range(MT):
                mc = mt // 2          # b chunk (2 m-tiles per chunk)
                j = mt % 2            # tile within chunk
                ps = psum_pool.tile([128, N_CODE], mybir.dt.float32)  # 4 banks
                for kp in range(KP):
                    for nt in range(NT):
                        nc.tensor.matmul(
                            ps[:, nt * 512 : (nt + 1) * 512],
                            b_tiles[mc][:, 2 * kp : 2 * kp + 2, j * 128 : (j + 1) * 128],
                            g_tiles[kp][:, :, nt * 512 : (nt + 1) * 512],
                            start=(kp == 0),
                            stop=(kp == KP - 1),
                            perf_mode=dr,
                        )
                mid = mids.tile([128, N_CODE], mybir.dt.uint16)
                nc.scalar.activation(mid, ps, mybir.ActivationFunctionType.Copy)
                ext_engines[mt % 2].tensor_scalar(
                    out=mid,
                    in0=mid,
                    scalar1=1,
                    scalar2=None,
                    op0=mybir.AluOpType.bitwise_and,
                )
                out_eng[mt].dma_start(out=c_view[mt], in_=mid)

    nc.finalize()
    return nc


def _get_nc():
    global _NC_CACHE
    if _NC_CACHE is None:
        _NC_CACHE = _build_bass()
    return _NC_CACHE


def _pack_inputs(b, G):
    b8 = np.asarray(b).astype(np.uint8)
    G8 = np.asarray(G).astype(np.uint8)
    # g[p, s, n], k = s*128 + p
    g_f8 = G8.reshape(KS, 128, N_CODE).transpose(1, 0, 2).astype(F8, order="C")
    bts = []
    for core in range(NCORES):
        sh = b8[core * M : (core + 1) * M]  # [M, K]
        # bt[p, c, s, j]: m = c*MCW + j, k = s*128 + p
        btc = sh.reshape(MC, MCW, KS, 128).transpose(3, 0, 2, 1)
        bts.append(btc.astype(F8, order="C"))
    return bts, g_f8


def kernel(b, G, trace=False, **run_kwargs):
    from concourse.bass_utils import run_bass_kernel_spmd

    nc = _get_nc()
    bts, g_f8 = _pack_inputs(b, G)
    in_maps = [{"bt": bts[i], "g": g_f8} for i in range(NCORES)]
    res = run_bass_kernel_spmd(
        nc, in_maps, core_ids=list(range(NCORES)), trace=trace, **run_kwargs
    )
    out = np.concatenate([res.results[i]["c"] for i in range(NCORES)], axis=0)
    out = out.astype(np.int32)
    if trace:
        kernel.last_results = res
    return out


kernel.last_results = None
